# revision 2
# baseline (speedup 1.0000x reference)
"""Trainium2 Bass kernel for a single-head causal attention module.

Problem (hardcoded): x [8, 2048, 1024] f32, W_Q/W_K/W_V [64, 1024] f32
    Q = x @ W_Q.T ; K = x @ W_K.T ; V = x @ W_V.T       (per batch)
    out = softmax(causal(Q @ K.T / sqrt(64))) @ V        -> [8, 2048, 64] f32

Sharding: batch dim across the 8 NeuronCores (data parallel, no collectives).

Per-core dataflow (all matmuls contract over the SBUF partition dim):
  1. PE-transpose x -> xT (d-major) since the QKV projections contract over d.
  2. One matmul per d-chunk computes Q^T and K^T together (W_Q^T | W_K^T packed
     along the stationary free dim), output j-major which is exactly what the
     scores matmul needs to contract over j.
  3. V^T likewise, then PE-transposed to V (s-major) with a ones column
     appended, so the P@V matmul also produces the softmax row-sums for free.
  4. Scores are built key-major (S^T) per 128-row key tile x 512-col q chunk;
     exp runs on ScalarE with the 1/sqrt(64) scale fused in; causal masking is
     an exact 0/1 upper-triangular multiply on diagonal blocks only (off-diag
     blocks above the diagonal are simply never computed).
  5. O^T accumulates in PSUM over key tiles; a final PE transpose + reciprocal
     scale by the ones-column row-sum normalizes and emits [128, 64] tiles.
"""

import os

import numpy as np

import concourse.bass as bass
import concourse.mybir as mybir
import concourse.tile as tile
from concourse import bacc
from concourse.bass_utils import run_bass_kernel_spmd
from concourse.masks import make_identity

B, S, D, J, P = 8, 2048, 1024, 64, 128
NCH = D // P  # 8 contraction chunks of 128
NSG = 4  # 512-wide s/q strips
SW = S // NSG  # 512
F32 = mybir.dt.float32

# Matmul input dtype: float32r streams 1 row/cycle (vs 4 for float32) at
# free-dim >= 256 on TRN2; numerics validated against the fp32 reference.
MM_DT = {
    "fp32": mybir.dt.float32,
    "fp32r": mybir.dt.float32r,
}[os.environ.get("ATTN_MM_DTYPE", "fp32r")]


def _mm(ap):
    return ap.bitcast(MM_DT) if MM_DT != F32 else ap


def _build():
    nc = bacc.Bacc("TRN2", debug=False)
    x = nc.dram_tensor("x", [S, D], F32, kind="ExternalInput").ap()
    w_q = nc.dram_tensor("W_Q", [J, D], F32, kind="ExternalInput").ap()
    w_k = nc.dram_tensor("W_K", [J, D], F32, kind="ExternalInput").ap()
    w_v = nc.dram_tensor("W_V", [J, D], F32, kind="ExternalInput").ap()
    out = nc.dram_tensor("out", [S, J], F32, kind="ExternalOutput").ap()

    AF = mybir.ActivationFunctionType

    with tile.TileContext(nc) as tc:
        from contextlib import ExitStack

        with ExitStack() as ctx:
            persist = ctx.enter_context(tc.tile_pool(name="persist", bufs=1))
            xsb_pool = ctx.enter_context(tc.tile_pool(name="xsb", bufs=2))
            wsb_pool = ctx.enter_context(tc.tile_pool(name="wsb", bufs=3))
            vtt_pool = ctx.enter_context(tc.tile_pool(name="vtt", bufs=2))
            pt_pool = ctx.enter_context(tc.tile_pool(name="ptp", bufs=4))
            otsb_pool = ctx.enter_context(tc.tile_pool(name="otsb", bufs=2))
            osb_pool = ctx.enter_context(tc.tile_pool(name="osb", bufs=3))
            rcp_pool = ctx.enter_context(tc.tile_pool(name="rcp", bufs=3))
            ps = ctx.enter_context(tc.tile_pool(name="ps", bufs=4, space="PSUM"))
            psot = ctx.enter_context(tc.tile_pool(name="psot", bufs=2, space="PSUM"))
            pssm = ctx.enter_context(tc.tile_pool(name="pssm", bufs=2, space="PSUM"))

            ident = persist.tile([P, P], F32, tag="ident")
            make_identity(nc, ident)
            # triu[p, f] = 1.0 iff f >= p  (valid: q_local >= k_local)
            triu = persist.tile([P, P], F32, tag="triu")
            nc.gpsimd.memset(triu, 1.0)
            nc.gpsimd.affine_select(
                out=triu,
                in_=triu,
                compare_op=mybir.AluOpType.is_ge,
                fill=0.0,
                base=0,
                pattern=[[1, P]],
                channel_multiplier=-1,
            )

            # W_Q^T | W_K^T packed along the stationary free dim; W_V^T alone.
            wqk_t = persist.tile([P, NCH, P], F32, tag="wqkt")
            wv_t = persist.tile([P, NCH, J], F32, tag="wvt")
            for wap, dst in (
                (w_q, wqk_t[:, :, 0:J]),
                (w_k, wqk_t[:, :, J:P]),
                (w_v, wv_t[:, :, 0:J]),
            ):
                wsb = wsb_pool.tile([J, D], F32, tag="wsb")
                nc.sync.dma_start(wsb, wap)
                psw = ps.tile([P, SW], F32, tag="ps512")
                for c in range(NCH):
                    nc.tensor.transpose(
                        psw[:, J * c : J * c + J],
                        wsb[:, P * c : P * c + P],
                        ident[0:J, 0:J],
                    )
                nc.any.tensor_copy(dst, psw.rearrange("p (c j) -> p c j", j=J))

            xt_s = [persist.tile([P, NCH, SW], F32, tag=f"xt{g}", name=f"xt{g}") for g in range(NSG)]
            qt_s = [persist.tile([J, SW], F32, tag=f"qt{g}", name=f"qt{g}") for g in range(NSG)]
            kt_s = [persist.tile([J, SW], F32, tag=f"kt{g}", name=f"kt{g}") for g in range(NSG)]
            vaug_s = [persist.tile([P, 4, 72], F32, tag=f"va{g}", name=f"va{g}") for g in range(NSG)]

            x_r = x.rearrange("(t p) d -> p t d", p=P)  # [128, 16, 1024]

            for sg in range(NSG):
                xs = xsb_pool.tile([P, 4, D], F32, tag="xs")
                nc.sync.dma_start(xs, x_r[:, 4 * sg : 4 * sg + 4, :])
                # transpose the strip: 8 chunks x 4 tiles of [128, 128]
                for dc in range(NCH):
                    pst = ps.tile([P, SW], F32, tag="ps512")
                    for k in range(4):
                        nc.tensor.transpose(
                            pst[:, P * k : P * k + P],
                            xs[:, k, P * dc : P * dc + P],
                            ident,
                        )
                    nc.any.tensor_copy(xt_s[sg][:, dc, :], pst)
                # Q^T | K^T projection (one matmul chain does both)
                psqk = ps.tile([P, SW], F32, tag="ps512")
                for dc in range(NCH):
                    nc.tensor.matmul(
                        psqk,
                        _mm(wqk_t[:, dc, :]),
                        _mm(xt_s[sg][:, dc, :]),
                        start=(dc == 0),
                        stop=(dc == NCH - 1),
                    )
                nc.any.tensor_copy(qt_s[sg], psqk[0:J])
                nc.any.tensor_copy(kt_s[sg], psqk[J:P])
                # V^T projection, then transpose into V|ones (s-major)
                psv = ps.tile([P, SW], F32, tag="ps512")
                for dc in range(NCH):
                    nc.tensor.matmul(
                        psv[0:J],
                        _mm(wv_t[:, dc, :]),
                        _mm(xt_s[sg][:, dc, :]),
                        start=(dc == 0),
                        stop=(dc == NCH - 1),
                    )
                vts = vtt_pool.tile([J, SW], F32, tag="vtt")
                nc.any.tensor_copy(vts, psv[0:J])
                nc.any.memset(vaug_s[sg][:, :, J : J + 1], 1.0)
                for k in range(4):
                    psv2 = pssm.tile([P, 72], F32, tag="small")
                    nc.tensor.transpose(
                        psv2[:, 0:J], vts[:, P * k : P * k + P], ident[0:J, 0:J]
                    )
                    nc.any.tensor_copy(vaug_s[sg][:, k, 0:J], psv2[:, 0:J])

            # scores -> exp -> mask -> P^T @ [V|1] accumulation, per q chunk
            for c in range(NSG):
                ot = psot.tile([J + 1, SW], F32, tag="ot")
                nt_c = 4 * c + 4  # key tiles 0 .. 4c+3 reach into chunk c
                for t in range(nt_c):
                    off = max(0, P * t - SW * c)
                    sgt, tl = t // 4, t % 4
                    pss = ps.tile([P, SW], F32, tag="ps512")
                    nc.tensor.matmul(
                        pss[:, off:SW],
                        _mm(kt_s[sgt][:, P * tl : P * tl + P]),
                        _mm(qt_s[c][:, off:SW]),
                        start=True,
                        stop=True,
                    )
                    ptc = pt_pool.tile([P, SW], F32, tag="ptc")
                    nc.scalar.activation(
                        ptc[:, off:SW], pss[:, off:SW], AF.Exp, scale=0.125
                    )
                    if t >= 4 * c:  # diagonal block: exact 0/1 causal mask
                        nc.vector.tensor_mul(
                            ptc[:, off : off + P], ptc[:, off : off + P], triu
                        )
                    nc.tensor.matmul(
                        ot[:, off:SW],
                        _mm(vaug_s[sgt][:, tl, 0 : J + 1]),
                        _mm(ptc[:, off:SW]),
                        start=(t == 0),
                        stop=(t == nt_c - 1),
                    )
                # normalize + transpose back to s-major, 4 tiles of [128, 64]
                otsb = otsb_pool.tile([J + 1, SW], F32, tag="otsb")
                nc.any.tensor_copy(otsb, ot)
                for k in range(4):
                    pso = pssm.tile([P, 72], F32, tag="small")
                    nc.tensor.transpose(
                        pso[:, 0 : J + 1],
                        otsb[:, P * k : P * k + P],
                        ident[0 : J + 1, 0 : J + 1],
                    )
                    rc = rcp_pool.tile([P, 1], F32, tag="rc")
                    nc.vector.reciprocal(rc, pso[:, J : J + 1])
                    o = osb_pool.tile([P, J], F32, tag="o")
                    nc.vector.tensor_scalar_mul(out=o, in0=pso[:, 0:J], scalar1=rc)
                    nc.sync.dma_start(out[SW * c + P * k : SW * c + P * k + P, :], o)

    nc.compile()
    return nc


_NC_CACHE = {}


def _get_nc():
    if "nc" not in _NC_CACHE:
        _NC_CACHE["nc"] = _build()
    return _NC_CACHE["nc"]


def kernel(x, W_Q, W_K, W_V):
    x = np.ascontiguousarray(np.asarray(x, dtype=np.float32))
    W_Q = np.ascontiguousarray(np.asarray(W_Q, dtype=np.float32))
    W_K = np.ascontiguousarray(np.asarray(W_K, dtype=np.float32))
    W_V = np.ascontiguousarray(np.asarray(W_V, dtype=np.float32))
    assert x.shape == (B, S, D)

    nc = _get_nc()
    in_maps = [
        {"x": np.ascontiguousarray(x[b]), "W_Q": W_Q, "W_K": W_K, "W_V": W_V}
        for b in range(B)
    ]
    res = run_bass_kernel_spmd(nc, in_maps, core_ids=list(range(B)))
    return np.stack([r["out"] for r in res.results], axis=0)


if __name__ == "__main__":
    rng = np.random.default_rng(0)
    inputs = {
        "x": rng.standard_normal((B, S, D), dtype=np.float32),
        "W_Q": (rng.random((J, D), dtype=np.float32) - 0.5) / 16.0,
        "W_K": (rng.random((J, D), dtype=np.float32) - 0.5) / 16.0,
        "W_V": (rng.random((J, D), dtype=np.float32) - 0.5) / 16.0,
    }
    got = kernel(**inputs)
    print("out", got.shape, got.dtype, np.abs(got).max())


# revision 5
# speedup vs baseline: 1.5919x; 1.5919x over previous
"""Trainium2 Bass kernel for a single-head causal attention module.

Problem (hardcoded): x [8, 2048, 1024] f32, W_Q/W_K/W_V [64, 1024] f32
    Q = x @ W_Q.T ; K = x @ W_K.T ; V = x @ W_V.T       (per batch)
    out = softmax(causal(Q @ K.T / sqrt(64))) @ V        -> [8, 2048, 64] f32

Sharding: batch dim across the 8 NeuronCores (data parallel, no collectives).

Per-core dataflow (all matmuls contract over the SBUF partition dim):
  1. PE-transpose x -> xT (d-major) since the QKV projections contract over d.
  2. One matmul per d-chunk computes Q^T and K^T together (W_Q^T | W_K^T packed
     along the stationary free dim), output j-major which is exactly what the
     scores matmul needs to contract over j.
  3. V^T likewise, then PE-transposed to V (s-major) with a ones column
     appended, so the P@V matmul also produces the softmax row-sums for free.
  4. Scores are built key-major (S^T) per 128-row key tile x 512-col q chunk;
     exp runs on ScalarE with the 1/sqrt(64) scale fused in; causal masking is
     an exact 0/1 upper-triangular multiply on diagonal blocks only (off-diag
     blocks above the diagonal are simply never computed).
  5. O^T accumulates in PSUM over key tiles; a final PE transpose + reciprocal
     scale by the ones-column row-sum normalizes and emits [128, 64] tiles.
"""

import os

import numpy as np

import concourse.bass as bass
import concourse.mybir as mybir
import concourse.tile as tile
from concourse import bacc
from concourse.bass_utils import run_bass_kernel_spmd
from concourse.masks import make_identity

B, S, D, J, P = 8, 2048, 1024, 64, 128
NCH = D // P  # 8 contraction chunks of 128
NSG = 4  # 512-wide s/q strips
SW = S // NSG  # 512
F32 = mybir.dt.float32

# Matmul input dtype: float32r streams 1 row/cycle (vs 4 for float32) at
# free-dim >= 256 on TRN2; numerics validated against the fp32 reference.
MM_DT = {
    "fp32": mybir.dt.float32,
    "fp32r": mybir.dt.float32r,
}[os.environ.get("ATTN_MM_DTYPE", "fp32r")]




def _build():
    nc = bacc.Bacc("TRN2", debug=False)
    x = nc.dram_tensor("x", [S, D], F32, kind="ExternalInput").ap()
    w_q = nc.dram_tensor("W_Q", [J, D], F32, kind="ExternalInput").ap()
    w_k = nc.dram_tensor("W_K", [J, D], F32, kind="ExternalInput").ap()
    w_v = nc.dram_tensor("W_V", [J, D], F32, kind="ExternalInput").ap()
    out = nc.dram_tensor("out", [S, J], F32, kind="ExternalOutput").ap()

    AF = mybir.ActivationFunctionType

    with tile.TileContext(nc) as tc:
        from contextlib import ExitStack

        with ExitStack() as ctx:
            persist = ctx.enter_context(tc.tile_pool(name="persist", bufs=1))
            xsb_pool = ctx.enter_context(tc.tile_pool(name="xsb", bufs=2))
            wsb_pool = ctx.enter_context(tc.tile_pool(name="wsb", bufs=3))
            vtt_pool = ctx.enter_context(tc.tile_pool(name="vtt", bufs=2))
            pt_pool = ctx.enter_context(tc.tile_pool(name="ptp", bufs=4))
            otsb_pool = ctx.enter_context(tc.tile_pool(name="otsb", bufs=2))
            osb_pool = ctx.enter_context(tc.tile_pool(name="osb", bufs=3))
            rcp_pool = ctx.enter_context(tc.tile_pool(name="rcp", bufs=3))
            ps = ctx.enter_context(tc.tile_pool(name="ps", bufs=4, space="PSUM"))
            psot = ctx.enter_context(tc.tile_pool(name="psot", bufs=2, space="PSUM"))
            pssm = ctx.enter_context(tc.tile_pool(name="pssm", bufs=2, space="PSUM"))

            ident = persist.tile([P, P], F32, tag="ident")
            make_identity(nc, ident)
            # triu[p, f] = 1.0 iff f >= p  (valid: q_local >= k_local)
            triu = persist.tile([P, P], F32, tag="triu")
            nc.gpsimd.memset(triu, 1.0)
            nc.gpsimd.affine_select(
                out=triu,
                in_=triu,
                compare_op=mybir.AluOpType.is_ge,
                fill=0.0,
                base=0,
                pattern=[[1, P]],
                channel_multiplier=-1,
            )

            # W_Q^T | W_K^T packed along the stationary free dim; W_V^T alone.
            wqk_t = persist.tile([P, NCH, P], MM_DT, tag="wqkt")
            wv_t = persist.tile([P, NCH, J], MM_DT, tag="wvt")
            for wap, dst in (
                (w_q, wqk_t[:, :, 0:J]),
                (w_k, wqk_t[:, :, J:P]),
                (w_v, wv_t[:, :, 0:J]),
            ):
                wsb = wsb_pool.tile([J, D], F32, tag="wsb")
                nc.sync.dma_start(wsb, wap)
                psw = ps.tile([P, SW], F32, tag="ps512")
                for c in range(NCH):
                    nc.tensor.transpose(
                        psw[:, J * c : J * c + J],
                        wsb[:, P * c : P * c + P],
                        ident[0:J, 0:J],
                    )
                nc.any.tensor_copy(dst, psw.rearrange("p (c j) -> p c j", j=J))

            xt_s = [persist.tile([P, NCH, SW], MM_DT, tag=f"xt{g}", name=f"xt{g}") for g in range(NSG)]
            qt_s = [persist.tile([J, SW], MM_DT, tag=f"qt{g}", name=f"qt{g}") for g in range(NSG)]
            kt_s = [persist.tile([J, SW], MM_DT, tag=f"kt{g}", name=f"kt{g}") for g in range(NSG)]
            vaug_s = [persist.tile([P, 4, 72], MM_DT, tag=f"va{g}", name=f"va{g}") for g in range(NSG)]

            x_r = x.rearrange("(t p) d -> p t d", p=P)  # [128, 16, 1024]

            for sg in range(NSG):
                xs = xsb_pool.tile([P, 4, D], F32, tag="xs")
                nc.sync.dma_start(xs, x_r[:, 4 * sg : 4 * sg + 4, :])
                # transpose the strip: 8 chunks x 4 tiles of [128, 128]
                for dc in range(NCH):
                    pst = ps.tile([P, SW], F32, tag="ps512")
                    for k in range(4):
                        nc.tensor.transpose(
                            pst[:, P * k : P * k + P],
                            xs[:, k, P * dc : P * dc + P],
                            ident,
                        )
                    nc.any.tensor_copy(xt_s[sg][:, dc, :], pst)
                # Q^T | K^T projection (one matmul chain does both)
                psqk = ps.tile([P, SW], F32, tag="ps512")
                for dc in range(NCH):
                    nc.tensor.matmul(
                        psqk,
                        (wqk_t[:, dc, :]),
                        (xt_s[sg][:, dc, :]),
                        start=(dc == 0),
                        stop=(dc == NCH - 1),
                    )
                nc.any.tensor_copy(qt_s[sg], psqk[0:J])
                nc.any.tensor_copy(kt_s[sg], psqk[J:P])
                # V^T projection, then transpose into V|ones (s-major)
                psv = ps.tile([P, SW], F32, tag="ps512")
                for dc in range(NCH):
                    nc.tensor.matmul(
                        psv[0:J],
                        (wv_t[:, dc, :]),
                        (xt_s[sg][:, dc, :]),
                        start=(dc == 0),
                        stop=(dc == NCH - 1),
                    )
                vts = vtt_pool.tile([J, SW], F32, tag="vtt")
                nc.any.tensor_copy(vts, psv[0:J])
                nc.any.memset(vaug_s[sg][:, :, J : J + 1].bitcast(F32), 1.0)
                for k in range(4):
                    psv2 = pssm.tile([P, 72], F32, tag="small")
                    nc.tensor.transpose(
                        psv2[:, 0:J], vts[:, P * k : P * k + P], ident[0:J, 0:J]
                    )
                    nc.any.tensor_copy(vaug_s[sg][:, k, 0:J], psv2[:, 0:J])

            # scores -> exp -> mask -> P^T @ [V|1] accumulation, per q chunk
            for c in range(NSG):
                ot = psot.tile([J + 1, SW], F32, tag="ot")
                nt_c = 4 * c + 4  # key tiles 0 .. 4c+3 reach into chunk c
                for t in range(nt_c):
                    off = max(0, P * t - SW * c)
                    sgt, tl = t // 4, t % 4
                    pss = ps.tile([P, SW], F32, tag="ps512")
                    nc.tensor.matmul(
                        pss[:, off:SW],
                        (kt_s[sgt][:, P * tl : P * tl + P]),
                        (qt_s[c][:, off:SW]),
                        start=True,
                        stop=True,
                    )
                    ptc = pt_pool.tile([P, SW], MM_DT, tag="ptc")
                    nc.scalar.activation(
                        ptc[:, off:SW], pss[:, off:SW], AF.Exp, scale=0.125
                    )
                    if t >= 4 * c:  # diagonal block: exact 0/1 causal mask
                        nc.vector.tensor_mul(
                            ptc[:, off : off + P], ptc[:, off : off + P], triu
                        )
                    nc.tensor.matmul(
                        ot[:, off:SW],
                        (vaug_s[sgt][:, tl, 0 : J + 1]),
                        (ptc[:, off:SW]),
                        start=(t == 0),
                        stop=(t == nt_c - 1),
                    )
                # normalize + transpose back to s-major, 4 tiles of [128, 64]
                otsb = otsb_pool.tile([J + 1, SW], F32, tag="otsb")
                nc.any.tensor_copy(otsb, ot)
                for k in range(4):
                    pso = pssm.tile([P, 72], F32, tag="small")
                    nc.tensor.transpose(
                        pso[:, 0 : J + 1],
                        otsb[:, P * k : P * k + P],
                        ident[0 : J + 1, 0 : J + 1],
                    )
                    rc = rcp_pool.tile([P, 1], F32, tag="rc")
                    nc.vector.reciprocal(rc, pso[:, J : J + 1])
                    o = osb_pool.tile([P, J], F32, tag="o")
                    nc.vector.tensor_scalar_mul(out=o, in0=pso[:, 0:J], scalar1=rc)
                    nc.sync.dma_start(out[SW * c + P * k : SW * c + P * k + P, :], o)

    nc.compile()
    return nc


_NC_CACHE = {}


def _get_nc():
    if "nc" not in _NC_CACHE:
        _NC_CACHE["nc"] = _build()
    return _NC_CACHE["nc"]


def kernel(x, W_Q, W_K, W_V):
    x = np.ascontiguousarray(np.asarray(x, dtype=np.float32))
    W_Q = np.ascontiguousarray(np.asarray(W_Q, dtype=np.float32))
    W_K = np.ascontiguousarray(np.asarray(W_K, dtype=np.float32))
    W_V = np.ascontiguousarray(np.asarray(W_V, dtype=np.float32))
    assert x.shape == (B, S, D)

    nc = _get_nc()
    in_maps = [
        {"x": np.ascontiguousarray(x[b]), "W_Q": W_Q, "W_K": W_K, "W_V": W_V}
        for b in range(B)
    ]
    res = run_bass_kernel_spmd(nc, in_maps, core_ids=list(range(B)))
    return np.stack([r["out"] for r in res.results], axis=0)


if __name__ == "__main__":
    rng = np.random.default_rng(0)
    inputs = {
        "x": rng.standard_normal((B, S, D), dtype=np.float32),
        "W_Q": (rng.random((J, D), dtype=np.float32) - 0.5) / 16.0,
        "W_K": (rng.random((J, D), dtype=np.float32) - 0.5) / 16.0,
        "W_V": (rng.random((J, D), dtype=np.float32) - 0.5) / 16.0,
    }
    got = kernel(**inputs)
    print("out", got.shape, got.dtype, np.abs(got).max())


# revision 6
# speedup vs baseline: 1.9390x; 1.2180x over previous
"""Trainium2 Bass kernel for a single-head causal attention module.

Problem (hardcoded): x [8, 2048, 1024] f32, W_Q/W_K/W_V [64, 1024] f32
    Q = x @ W_Q.T ; K = x @ W_K.T ; V = x @ W_V.T       (per batch)
    out = softmax(causal(Q @ K.T / sqrt(64))) @ V        -> [8, 2048, 64] f32

Sharding: batch dim across the 8 NeuronCores (data parallel, no collectives).

Per-core dataflow (all matmuls contract over the SBUF partition dim):
  1. PE-transpose x -> xT (d-major) since the QKV projections contract over d.
  2. One matmul per d-chunk computes Q^T and K^T together (W_Q^T | W_K^T packed
     along the stationary free dim), output j-major which is exactly what the
     scores matmul needs to contract over j.
  3. V^T likewise, then PE-transposed to V (s-major) with a ones column
     appended, so the P@V matmul also produces the softmax row-sums for free.
  4. Scores are built key-major (S^T) per 128-row key tile x 512-col q chunk;
     exp runs on ScalarE with the 1/sqrt(64) scale fused in; causal masking is
     an exact 0/1 upper-triangular multiply on diagonal blocks only (off-diag
     blocks above the diagonal are simply never computed).
  5. O^T accumulates in PSUM over key tiles; a final PE transpose + reciprocal
     scale by the ones-column row-sum normalizes and emits [128, 64] tiles.
"""

import os

import numpy as np

import concourse.bass as bass
import concourse.mybir as mybir
import concourse.tile as tile
from concourse import bacc
from concourse.bass_utils import run_bass_kernel_spmd
from concourse.masks import make_identity

B, S, D, J, P = 8, 2048, 1024, 64, 128
NCH = D // P  # 8 contraction chunks of 128
NSG = 4  # 512-wide s/q strips
SW = S // NSG  # 512
F32 = mybir.dt.float32

# Matmul input dtype: float32r streams 1 row/cycle (vs 4 for float32) at
# free-dim >= 256 on TRN2; numerics validated against the fp32 reference.
MM_DT = {
    "fp32": mybir.dt.float32,
    "fp32r": mybir.dt.float32r,
    "bf16": mybir.dt.bfloat16,
}[os.environ.get("ATTN_MM_DTYPE", "fp32r")]




def _build():
    nc = bacc.Bacc("TRN2", debug=False)
    x = nc.dram_tensor("x", [S, D], F32, kind="ExternalInput").ap()
    w_q = nc.dram_tensor("W_Q", [J, D], F32, kind="ExternalInput").ap()
    w_k = nc.dram_tensor("W_K", [J, D], F32, kind="ExternalInput").ap()
    w_v = nc.dram_tensor("W_V", [J, D], F32, kind="ExternalInput").ap()
    out = nc.dram_tensor("out", [S, J], F32, kind="ExternalOutput").ap()

    AF = mybir.ActivationFunctionType

    with tile.TileContext(nc) as tc:
        from contextlib import ExitStack

        with ExitStack() as ctx:
            persist = ctx.enter_context(tc.tile_pool(name="persist", bufs=1))
            xsb_pool = ctx.enter_context(tc.tile_pool(name="xsb", bufs=2))
            wsb_pool = ctx.enter_context(tc.tile_pool(name="wsb", bufs=3))
            vtt_pool = ctx.enter_context(tc.tile_pool(name="vtt", bufs=2))
            pt_pool = ctx.enter_context(tc.tile_pool(name="ptp", bufs=4))
            otsb_pool = ctx.enter_context(tc.tile_pool(name="otsb", bufs=2))
            osb_pool = ctx.enter_context(tc.tile_pool(name="osb", bufs=3))
            rcp_pool = ctx.enter_context(tc.tile_pool(name="rcp", bufs=3))
            ps = ctx.enter_context(tc.tile_pool(name="ps", bufs=4, space="PSUM"))
            psot = ctx.enter_context(tc.tile_pool(name="psot", bufs=2, space="PSUM"))
            pssm = ctx.enter_context(tc.tile_pool(name="pssm", bufs=2, space="PSUM"))

            ident = persist.tile([P, P], F32, tag="ident")
            make_identity(nc, ident)
            # triu[p, f] = 1.0 iff f >= p  (valid: q_local >= k_local)
            triu = persist.tile([P, P], F32, tag="triu")
            nc.gpsimd.memset(triu, 1.0)
            nc.gpsimd.affine_select(
                out=triu,
                in_=triu,
                compare_op=mybir.AluOpType.is_ge,
                fill=0.0,
                base=0,
                pattern=[[1, P]],
                channel_multiplier=-1,
            )

            # W_Q^T | W_K^T packed along the stationary free dim; W_V^T alone.
            wqk_t = persist.tile([P, NCH, P], MM_DT, tag="wqkt")
            wv_t = persist.tile([P, NCH, J], MM_DT, tag="wvt")
            for wap, dst in (
                (w_q, wqk_t[:, :, 0:J]),
                (w_k, wqk_t[:, :, J:P]),
                (w_v, wv_t[:, :, 0:J]),
            ):
                wsb = wsb_pool.tile([J, D], F32, tag="wsb")
                nc.sync.dma_start(wsb, wap)
                psw = ps.tile([P, SW], F32, tag="ps512")
                for c in range(NCH):
                    nc.tensor.transpose(
                        psw[:, J * c : J * c + J],
                        wsb[:, P * c : P * c + P],
                        ident[0:J, 0:J],
                    )
                nc.any.tensor_copy(dst, psw.rearrange("p (c j) -> p c j", j=J))

            xt_s = [persist.tile([P, NCH, SW], MM_DT, tag=f"xt{g}", name=f"xt{g}") for g in range(NSG)]
            qt_s = [persist.tile([J, SW], MM_DT, tag=f"qt{g}", name=f"qt{g}") for g in range(NSG)]
            kt_s = [persist.tile([J, SW], MM_DT, tag=f"kt{g}", name=f"kt{g}") for g in range(NSG)]
            vaug_s = [persist.tile([P, 4, 72], MM_DT, tag=f"va{g}", name=f"va{g}") for g in range(NSG)]

            x_r = x.rearrange("(t p) d -> p t d", p=P)  # [128, 16, 1024]

            for sg in range(NSG):
                xs = xsb_pool.tile([P, 4, D], F32, tag="xs")
                nc.sync.dma_start(xs, x_r[:, 4 * sg : 4 * sg + 4, :])
                # transpose the strip: 8 chunks x 4 tiles of [128, 128]
                for dc in range(NCH):
                    pst = ps.tile([P, SW], F32, tag="ps512")
                    for k in range(4):
                        nc.tensor.transpose(
                            pst[:, P * k : P * k + P],
                            xs[:, k, P * dc : P * dc + P],
                            ident,
                        )
                    nc.any.tensor_copy(xt_s[sg][:, dc, :], pst)
                # Q^T | K^T projection (one matmul chain does both)
                psqk = ps.tile([P, SW], F32, tag="ps512")
                for dc in range(NCH):
                    nc.tensor.matmul(
                        psqk,
                        (wqk_t[:, dc, :]),
                        (xt_s[sg][:, dc, :]),
                        start=(dc == 0),
                        stop=(dc == NCH - 1),
                    )
                nc.any.tensor_copy(qt_s[sg], psqk[0:J])
                nc.any.tensor_copy(kt_s[sg], psqk[J:P])
                # V^T projection, then transpose into V|ones (s-major)
                psv = ps.tile([P, SW], F32, tag="ps512")
                for dc in range(NCH):
                    nc.tensor.matmul(
                        psv[0:J],
                        (wv_t[:, dc, :]),
                        (xt_s[sg][:, dc, :]),
                        start=(dc == 0),
                        stop=(dc == NCH - 1),
                    )
                vts = vtt_pool.tile([J, SW], F32, tag="vtt")
                nc.any.tensor_copy(vts, psv[0:J])
                nc.any.memset(vaug_s[sg][:, :, J : J + 1] if MM_DT != mybir.dt.float32r else vaug_s[sg][:, :, J : J + 1].bitcast(F32), 1.0)
                for k in range(4):
                    psv2 = pssm.tile([P, 72], F32, tag="small")
                    nc.tensor.transpose(
                        psv2[:, 0:J], vts[:, P * k : P * k + P], ident[0:J, 0:J]
                    )
                    nc.any.tensor_copy(vaug_s[sg][:, k, 0:J], psv2[:, 0:J])

            # scores -> exp -> mask -> P^T @ [V|1] accumulation, per q chunk
            for c in range(NSG):
                ot = psot.tile([J + 1, SW], F32, tag="ot")
                nt_c = 4 * c + 4  # key tiles 0 .. 4c+3 reach into chunk c
                for t in range(nt_c):
                    off = max(0, P * t - SW * c)
                    sgt, tl = t // 4, t % 4
                    pss = ps.tile([P, SW], F32, tag="ps512")
                    nc.tensor.matmul(
                        pss[:, off:SW],
                        (kt_s[sgt][:, P * tl : P * tl + P]),
                        (qt_s[c][:, off:SW]),
                        start=True,
                        stop=True,
                    )
                    ptc = pt_pool.tile([P, SW], MM_DT, tag="ptc")
                    nc.scalar.activation(
                        ptc[:, off:SW], pss[:, off:SW], AF.Exp, scale=0.125
                    )
                    if t >= 4 * c:  # diagonal block: exact 0/1 causal mask
                        nc.vector.tensor_mul(
                            ptc[:, off : off + P], ptc[:, off : off + P], triu
                        )
                    nc.tensor.matmul(
                        ot[:, off:SW],
                        (vaug_s[sgt][:, tl, 0 : J + 1]),
                        (ptc[:, off:SW]),
                        start=(t == 0),
                        stop=(t == nt_c - 1),
                    )
                # normalize + transpose back to s-major, 4 tiles of [128, 64]
                otsb = otsb_pool.tile([J + 1, SW], F32, tag="otsb")
                nc.any.tensor_copy(otsb, ot)
                for k in range(4):
                    pso = pssm.tile([P, 72], F32, tag="small")
                    nc.tensor.transpose(
                        pso[:, 0 : J + 1],
                        otsb[:, P * k : P * k + P],
                        ident[0 : J + 1, 0 : J + 1],
                    )
                    rc = rcp_pool.tile([P, 1], F32, tag="rc")
                    nc.vector.reciprocal(rc, pso[:, J : J + 1])
                    o = osb_pool.tile([P, J], F32, tag="o")
                    nc.vector.tensor_scalar_mul(out=o, in0=pso[:, 0:J], scalar1=rc)
                    nc.sync.dma_start(out[SW * c + P * k : SW * c + P * k + P, :], o)

    nc.compile()
    return nc


_NC_CACHE = {}


def _get_nc():
    if "nc" not in _NC_CACHE:
        _NC_CACHE["nc"] = _build()
    return _NC_CACHE["nc"]


def kernel(x, W_Q, W_K, W_V):
    x = np.ascontiguousarray(np.asarray(x, dtype=np.float32))
    W_Q = np.ascontiguousarray(np.asarray(W_Q, dtype=np.float32))
    W_K = np.ascontiguousarray(np.asarray(W_K, dtype=np.float32))
    W_V = np.ascontiguousarray(np.asarray(W_V, dtype=np.float32))
    assert x.shape == (B, S, D)

    nc = _get_nc()
    in_maps = [
        {"x": np.ascontiguousarray(x[b]), "W_Q": W_Q, "W_K": W_K, "W_V": W_V}
        for b in range(B)
    ]
    res = run_bass_kernel_spmd(nc, in_maps, core_ids=list(range(B)))
    return np.stack([r["out"] for r in res.results], axis=0)


if __name__ == "__main__":
    rng = np.random.default_rng(0)
    inputs = {
        "x": rng.standard_normal((B, S, D), dtype=np.float32),
        "W_Q": (rng.random((J, D), dtype=np.float32) - 0.5) / 16.0,
        "W_K": (rng.random((J, D), dtype=np.float32) - 0.5) / 16.0,
        "W_V": (rng.random((J, D), dtype=np.float32) - 0.5) / 16.0,
    }
    got = kernel(**inputs)
    print("out", got.shape, got.dtype, np.abs(got).max())
